# revision 26
# baseline (speedup 1.0000x reference)
"""Trainium2 Bass kernel for sliding-window causal MHA with RoPE + ALiBi.

Sharding: 8 cores = 4 batches x 2 head-sets. Head-sets interleave parity
(core parity p takes global heads p, p+2, ..., p+14) so both per-core
programs have identical attention tile counts after ALiBi-decay window
truncation (steep-slope heads attend far fewer than W keys).

Per-core device program, all matmuls fp16:
  A: v-proj, n-outer accumulation (PE stays fed during the x/wv DMA fill)
  B: q/k-proj + RoPE (Act evac w/ bias, DVE fp16 rope at 2x rate)
  C: per query-group gi (128 queries), two half-passes of 4 heads:
     scores for a descending-j0 span -> one Act exp -> one DVE mask-mul
     (expb master tile: ALiBi weight * window mask, contiguous slice),
     then PV + ones-sums accumulation packed 4 heads/bank, DVE
     reciprocal + normalize. Truncated j-span per head slot via T_PAT.
  D: out-proj interleaved one query-group behind C (fills exp latency),
     partial over the head set; host sums partials + bo + wo@bv.
"""
import sys
sys.path.insert(0, '/opt/trn_rl_repo')
from contextlib import ExitStack

import numpy as np
import concourse.bass as bass
import concourse.bacc as bacc
import concourse.mybir as mybir
import concourse.tile as tile

L, N, C, H, D, W = 1024, 4, 2048, 16, 128, 512
HPC = 8                       # head slots per core
GD = HPC * D                  # 1024 head-dims per core
SCALE = 1.0 / float(np.sqrt(D))
F32 = mybir.dt.float32
F16 = mybir.dt.float16
AF = mybir.ActivationFunctionType
NT_C = C // 128               # 16 contraction tiles over embed dim
NT_HD = GD // 128             # 8 head tiles (1 head each, D=128)
NT_T = L // 128               # 8 token tiles
MASK_W = 640                  # expb master width: y = di + (i0-j0), T<=512
# Truncated window per head slot (parity-max so both core programs match).
# Slot s holds global head 2s+p; slope(s,p)=2^{-(2s+p+1)/2}. T chosen so
# dropped softmax mass <~ e^-8 relative even for the shallower parity.
T_PAT = [32, 64, 128, 128, 256, 512, 512, 512]


def jtiles(s, gi):
    """Descending j0 list for head-slot s, query group [128*gi, 128*gi+128)."""
    i0 = gi * 128
    lo = max(0, i0 - T_PAT[s]) // 128 * 128
    return list(range(i0, lo - 1, -128))


def chunks(lst, n=4):
    return [lst[i:i + n] for i in range(0, len(lst), n)]


def emit(tc, t):
    nc = tc.nc
    cpool = tc.alloc_tile_pool(name="const", bufs=1, side="left")
    cos2 = cpool.tile([128, L], F16, tag="cos2")
    sin2 = cpool.tile([128, L], F16, tag="sin2")
    bq_s = cpool.tile([128, NT_HD], F32, tag="bq")
    bk_s = cpool.tile([128, NT_HD], F32, tag="bk")
    ones = cpool.tile([128, 128], F16, tag="ones")

    # long-lived (left stack): v tiles, then q/k tiles
    vp = tc.alloc_tile_pool(name="vp", bufs=1, side="left")
    vts = [vp.tile([128, GD], F16, tag=f"v{tt}", name=f"v{tt}") for tt in range(NT_T)]

    # single PSUM pool: 8 bank-tags handed across phases with zero
    # pool-transition stalls (WAR deps per tag do the synchronization)
    psp = tc.alloc_tile_pool(name="psp", bufs=1, space="PSUM")

    def bank(i, width=512):
        return psp.tile([128, width], F32, tag=f"b{i}", name=f"b{i}")

    # ---------------- phase A: v-proj (n-outer, 4 passes of 4 groups) -------
    # fill bandwidth: x on the SP queue, wv on the DVE queue in parallel so
    # (x_n, wv_n) pairs arrive faster than the PE consumes them
    xp = tc.alloc_tile_pool(name="xp", bufs=1, side="right")
    expb = [cpool.tile([128, MASK_W], F16, tag=f"eb{s}", name=f"eb{s}")
            for s in range(HPC)]
    wots = [cpool.tile([128, C], F16, tag=f"wo{s}", name=f"wo{s}")
            for s in range(NT_HD)]
    w0p = tc.alloc_tile_pool(name="w0p", bufs=1, side="right")
    # x and wv live interleaved in 4 n-quarter tiles [128, 4 n, 2, 1024]
    # (which=0 -> x tokens, which=1 -> wv head-dims); one DMA delivers the
    # (x token-half, wv gd-half) pair each A-pass-1 n-tile needs, in chunks
    # sized so arrivals outpace PE consumption
    xwv = [xp.tile([128, 4, 2, L], F16, tag=f"x{q}", name=f"x{q}")
           for q in range(4)]

    def xsl(n, a, b):
        return xwv[n // 4][:, n % 4, 0, a:b]

    def wsl(n, a, b):
        return xwv[n // 4][:, n % 4, 1, a:b]

    def fill_chunk(q, ja, jb):
        nc.sync.dma_start(xwv[q][:, ja:jb, :, 0:768],
                          t["xwv"][q][:, ja:jb, :, 0:768])

    if True:
        # pass 1 covers 6 of 8 groups, so PE demand per n-tile (1278ns)
        # far exceeds the chunk supply rate (~910ns): robust to jitter
        for j in range(4):
            fill_chunk(0, j, j + 1)
        for q in range(1, 4):
            fill_chunk(q, 0, 2)
            fill_chunk(q, 2, 4)
        for q in range(4):
            nc.sync.dma_start(xwv[q][:, :, 0, 768:1024],
                              t["xwv"][q][:, :, 0, 768:1024])
        for q in range(4):
            nc.sync.dma_start(xwv[q][:, :, 1, 768:1024],
                              t["xwv"][q][:, :, 1, 768:1024])
        # small consts after the fill-critical stream
        nc.sync.dma_start(cos2[:], t["cos2"][:])
        nc.sync.dma_start(sin2[:], t["sin2"][:])
        nc.sync.dma_start(bq_s[:], t["bq"][:])
        nc.sync.dma_start(bk_s[:], t["bk"][:])
        nc.sync.dma_start(ones[:], t["ones"][:])
        # m=0 q/k weights ahead of the expb/wo bulk so B can start on time
        wt0 = {}
        for wname in ("wq", "wk"):
            wt0[wname] = w0p.tile([128, C], F16, tag=f"{wname}0", name=f"{wname}0")
            nc.sync.dma_start(wt0[wname][:], t[wname][0])
        for s in range(HPC):
            nc.sync.dma_start(expb[s][:], t["expb"][s])
        for s in range(NT_HD):
            nc.sync.dma_start(wots[s][:], t["wo"][s])
        for i2 in range(2):
            for tts, bs in ((range(0, 6), (0, 1, 2, 3, 4, 5)),
                            (range(6, 8), (6, 7))):
                groups = [(tt, i2) for tt in tts]
                pss = [bank(bs[gidx]) for gidx in range(len(groups))]
                for n in range(NT_C):
                    for gidx, (tt, i2g) in enumerate(groups):
                        nc.tensor.matmul(
                            pss[gidx][:],
                            xsl(n, tt * 128, (tt + 1) * 128),
                            wsl(n, i2g * 512, (i2g + 1) * 512),
                            start=(n == 0), stop=(n == NT_C - 1))
                for gidx, (tt, i2g) in enumerate(groups):
                    nc.scalar.activation(
                        vts[tt][:, i2g * 512:(i2g + 1) * 512], pss[gidx][:],
                        AF.Identity, scale=1.0)

    # ---------------- phase B: q/k-proj + rope ----------------
    qkp = tc.alloc_tile_pool(name="qkp", bufs=1, side="left")
    qts = [qkp.tile([128, L], F16, tag=f"q{m}", name=f"q{m}") for m in range(NT_HD)]
    kts = [qkp.tile([128, L], F16, tag=f"k{m}", name=f"k{m}") for m in range(NT_HD)]
    g0p = tc.alloc_tile_pool(name="g0p", bufs=1, side="left")
    awp = tc.alloc_tile_pool(name="awp", bufs=2, side="left")
    cw = tc.alloc_tile_pool(name="cw", bufs=3, side="left")
    pre_pts = {0: {}, 1: {}}
    precnt = [0]
    aw_tiles = {}
    acc01 = {}

    def pv_sums(s, ls, attn2, sums2, pts):
        tiles = [(j0, pT, ci)
                 for chunk, pT in pts[s]
                 for ci, j0 in enumerate(chunk)]
        for ti, (j0, pT, ci) in enumerate(tiles):
            nc.tensor.matmul(
                attn2[:, ls * 128:(ls + 1) * 128],
                vts[j0 // 128][:, s * 128:(s + 1) * 128],
                pT[:, ci * 128:(ci + 1) * 128],
                start=(ti == 0), stop=(ti == len(tiles) - 1))
            nc.tensor.matmul(
                sums2[:, ls * 128:(ls + 1) * 128],
                ones[:],
                pT[:, ci * 128:(ci + 1) * 128],
                start=(ti == 0), stop=(ti == len(tiles) - 1))

    def quarter_norm(gi, qp, attn2, sums2):
        rec = cw.tile([128, 256], F32, tag="rec", name="rec")
        nc.vector.reciprocal(rec[:], sums2)
        awq = awp.tile([128, 256], F16, tag=f"aw{qp}", name=f"aw{qp}")
        nc.vector.tensor_mul(awq[:], attn2, rec[:])
        aw_tiles.setdefault(gi, [None] * 4)[qp] = awq

    def gi0_pv(qp):
        if "ab" not in acc01:
            acc01["ab"] = bank(2)
            acc01["sb"] = bank(3)
        hsl = slice((qp % 2) * 256, (qp % 2) * 256 + 256)
        attn2, sums2 = acc01["ab"][:, hsl], acc01["sb"][:, hsl]
        for ls, s in enumerate((2 * qp, 2 * qp + 1)):
            pv_sums(s, ls, attn2, sums2, pre_pts[0])
        quarter_norm(0, qp, attn2, sums2)

    def pre_scores(s, gi):
        # gi=0/1 scores emitted during B so their exp chains hide under B's
        # matmuls; banks 6/7 are free of B's rotation (0..5)
        i0 = gi * 128
        pre_pts[gi][s] = []
        for chunk in chunks(jtiles(s, gi)):
            ck = len(chunk)
            s_ps = bank(6 + precnt[0] % 2, width=ck * 128)
            precnt[0] += 1
            for ci, j0 in enumerate(chunk):
                nc.tensor.matmul(s_ps[:, ci * 128:(ci + 1) * 128],
                                 kts[s][:, j0:j0 + 128], qts[s][:, i0:i0 + 128],
                                 start=True, stop=True)
            e = g0p.tile([128, ck * 128], F16, tag=f"pe{precnt[0] % 2}",
                         name=f"pe{precnt[0] % 2}")
            nc.scalar.activation(e[:], s_ps[:], AF.Exp, scale=SCALE)
            pT = g0p.tile([128, ck * 128], F16, tag=f"pT{gi}{s}", name=f"pT{gi}{s}")
            c0 = (i0 - chunk[0]) // 128
            nc.vector.tensor_mul(pT[:], e[:], expb[s][:, c0 * 128:(c0 + ck) * 128])
            pre_pts[gi][s].append((chunk, pT))

    bcnt = [0]
    with tc.tile_pool(name="ws", bufs=2, side="right") as ws, \
         tc.tile_pool(name="rp", bufs=3, side="right") as rp:
        for m in range(NT_HD):
            for wname, dst, bias_s in (("wq", qts, bq_s), ("wk", kts, bk_s)):
                if m == 0:
                    wt = wt0[wname]
                else:
                    wt = ws.tile([128, C], F16, tag="wqk", name="wqk")
                    nc.sync.dma_start(wt[:], t[wname][m])
                for i2 in range(2):
                    ps = bank((0, 1, 4, 5)[bcnt[0] % 4])
                    bcnt[0] += 1
                    for n in range(NT_C):
                        nc.tensor.matmul(
                            ps[:],
                            wt[:, n * 128:(n + 1) * 128],
                            xsl(n, i2 * 512, (i2 + 1) * 512),
                            start=(n == 0), stop=(n == NT_C - 1))
                    csl = slice(i2 * 512, (i2 + 1) * 512)
                    qw = rp.tile([128, 512], F16, tag="qw", name="qw")
                    nc.scalar.activation(
                        qw[:], ps[:],
                        AF.Identity, bias=bias_s[:, m:m + 1], scale=1.0)
                    # rope: dst = qw*cos2 + swap_halves(qw)*sin2, all fp16
                    rot = rp.tile([128, 512], F16, tag="rot", name="rot")
                    nc.vector.tensor_copy(rot[0:64, :], qw[64:128, :])
                    nc.vector.tensor_copy(rot[64:128, :], qw[0:64, :])
                    t1 = rp.tile([128, 512], F16, tag="t1", name="t1")
                    nc.vector.tensor_mul(t1[:], qw[:], cos2[:, csl])
                    nc.vector.tensor_mul(rot[:], rot[:], sin2[:, csl])
                    nc.vector.tensor_add(dst[m][:, csl], t1[:], rot[:])
            if m >= 1:
                pre_scores(m - 1, 0)
                pre_scores(m - 1, 1)
            if m in (3, 5, 7):
                gi0_pv((m - 3) // 2)
    w0p.release()
    xp.release()

    # ---------------- phase C+D: attention + out-proj, interleaved ----------
    sccnt = [0]
    dcnt = [0]
    with tc.tile_pool(name="og", bufs=3, side="right") as og:

        def d_chain(tt, cc):
            ps = bank(6 + dcnt[0] % 2)
            dcnt[0] += 1
            for hh in range(NT_HD):
                aw = aw_tiles[tt][hh // 2]
                ls = hh % 2
                nc.tensor.matmul(
                    ps[:],
                    aw[:, ls * 128:(ls + 1) * 128],
                    wots[hh][:, cc * 512:(cc + 1) * 512],
                    start=(hh == 0), stop=(hh == NT_HD - 1))
            o = og.tile([128, 512], F32, tag="o", name="o")
            nc.scalar.activation(o[:], ps[:], AF.Identity, scale=1.0)
            nc.sync.dma_start(
                t["out"][tt * 128:(tt + 1) * 128, cc * 512:(cc + 1) * 512], o[:])

        def scores_for(s, gi, pts, banks=(0, 1, 4, 5)):
            i0 = gi * 128
            pts[s] = []
            for chunk in chunks(jtiles(s, gi)):
                ck = len(chunk)
                s_ps = bank(banks[sccnt[0] % len(banks)], width=ck * 128)
                sccnt[0] += 1
                for ci, j0 in enumerate(chunk):
                    nc.tensor.matmul(
                        s_ps[:, ci * 128:(ci + 1) * 128],
                        kts[s][:, j0:j0 + 128],
                        qts[s][:, i0:i0 + 128],
                        start=True, stop=True)
                e = cw.tile([128, ck * 128], F16, tag="e", name="e")
                nc.scalar.activation(e[:], s_ps[:], AF.Exp, scale=SCALE)
                pT = cw.tile([128, ck * 128], F16, tag="pT", name="pT")
                c0 = (i0 - chunk[0]) // 128
                nc.vector.tensor_mul(
                    pT[:], e[:], expb[s][:, c0 * 128:(c0 + ck) * 128])
                pts[s].append((chunk, pT))



        def gi1_qp(qp, with_d=True):
            hsl = slice((qp % 2) * 256, (qp % 2) * 256 + 256)
            attn2, sums2 = acc01["ab"][:, hsl], acc01["sb"][:, hsl]
            if with_d:
                d_chain(0, qp)
            for ls, s in enumerate((2 * qp, 2 * qp + 1)):
                pv_sums(s, ls, attn2, sums2, pre_pts[1])
            quarter_norm(1, qp, attn2, sums2)

        # C start: gi0-qp3 and slot-7 pre-scores were not possible during B
        # (rope(7) lands after B's last chain); interleave them behind gi1's
        # first quarters so the PE never waits on them directly
        gi1_qp(0, with_d=False)
        gi1_qp(1, with_d=False)
        pre_scores(7, 0)
        pre_scores(7, 1)
        gi0_pv(3)
        gi1_qp(2, with_d=False)
        d_chain(0, 0)
        gi1_qp(3, with_d=False)
        d_chain(0, 1)
        d_chain(0, 2)
        d_chain(0, 3)

        # quarter-passes of 2 heads; even/odd quarters use the two halves of
        # one attn bank (b2) and one sums bank (b3) -- groups in a bank stay
        # sequential, never concurrently open -- freeing b4/b5 for scores
        for gi in range(2, NT_T):
            ab = bank(2)
            sb = bank(3)
            for qp in range(4):
                hpair = (2 * qp, 2 * qp + 1)
                hsl = slice((qp % 2) * 256, (qp % 2) * 256 + 256)
                attn2 = ab[:, hsl]
                sums2 = sb[:, hsl]
                pts = {}
                scores_for(hpair[0], gi, pts)
                scores_for(hpair[1], gi, pts)
                d_chain(gi - 1, qp)
                for ls, s in enumerate(hpair):
                    pv_sums(s, ls, attn2, sums2, pts)
                quarter_norm(gi, qp, attn2, sums2)
        for cc in range(3):
            d_chain(NT_T - 1, cc)
        # final chain split in two halves so the tail evac/DMA starts earlier
        for half in range(2):
            ps = bank(6 + half)
            for hh in range(NT_HD):
                aw = aw_tiles[NT_T - 1][hh // 2]
                nc.tensor.matmul(
                    ps[:, 0:256],
                    aw[:, (hh % 2) * 128:(hh % 2 + 1) * 128],
                    wots[hh][:, 1536 + half * 256:1536 + half * 256 + 256],
                    start=(hh == 0), stop=(hh == NT_HD - 1))
            o = og.tile([128, 256], F32, tag="o", name="o")
            nc.scalar.activation(o[:], ps[:, 0:256], AF.Identity, scale=1.0)
            nc.sync.dma_start(
                t["out"][(NT_T - 1) * 128:NT_T * 128,
                         1536 + half * 256:1536 + half * 256 + 256], o[:])

    psp.release()
    cw.release()
    awp.release()
    g0p.release()
    qkp.release()
    vp.release()
    cpool.release()


def build_nc(enable_asserts=False):
    nc = bacc.Bacc("TRN2", target_bir_lowering=False, debug=False,
                   enable_asserts=enable_asserts, num_devices=8)
    t = {}
    t["xwv"] = nc.dram_tensor("xwv", [4, 128, 4, 2, 1024], F16, kind="ExternalInput").ap()
    t["wq"] = nc.dram_tensor("wq", [NT_HD, 128, C], F16, kind="ExternalInput").ap()
    t["wk"] = nc.dram_tensor("wk", [NT_HD, 128, C], F16, kind="ExternalInput").ap()
    t["wo"] = nc.dram_tensor("wo", [NT_HD, 128, C], F16, kind="ExternalInput").ap()
    t["cos2"] = nc.dram_tensor("cos2", [128, L], F16, kind="ExternalInput").ap()
    t["sin2"] = nc.dram_tensor("sin2", [128, L], F16, kind="ExternalInput").ap()
    t["bq"] = nc.dram_tensor("bq", [128, NT_HD], F32, kind="ExternalInput").ap()
    t["bk"] = nc.dram_tensor("bk", [128, NT_HD], F32, kind="ExternalInput").ap()
    t["expb"] = nc.dram_tensor("expb", [HPC, 128, MASK_W], F16, kind="ExternalInput").ap()
    t["ones"] = nc.dram_tensor("ones", [128, 128], F16, kind="ExternalInput").ap()
    t["out"] = nc.dram_tensor("out", [L, C], F32, kind="ExternalOutput").ap()
    with tile.TileContext(nc) as tc:
        emit(tc, t)
    nc.compile()
    return nc


def marshal(inputs):
    x = np.asarray(inputs["x"], np.float32)
    wq = np.asarray(inputs["wq"], np.float32)
    wkv = np.asarray(inputs["wkv"], np.float32)
    wo = np.asarray(inputs["wo"], np.float32)
    bq = np.asarray(inputs["bq"], np.float32)
    bkv = np.asarray(inputs["bkv"], np.float32)
    alibi = np.asarray(inputs["alibi_slopes"], np.float32)
    wk_full, wv_full = wkv[:C], wkv[C:]
    bk_full = bkv[:C]

    perm = np.concatenate([np.arange(0, D, 2), np.arange(1, D, 2)])

    t_abs = np.arange(W, W + L, dtype=np.float64)
    inv = 1.0 / (10000.0 ** (np.arange(0, D, 2, dtype=np.float64) / D))
    fr = np.outer(t_abs, inv)
    cosT = np.cos(fr).T.astype(np.float32)
    sinT = np.sin(fr).T.astype(np.float32)
    cos2 = np.ascontiguousarray(np.concatenate([cosT, cosT], 0)).astype(np.float16)
    sin2 = np.ascontiguousarray(np.concatenate([-sinT, sinT], 0)).astype(np.float16)

    dj = np.arange(128)[:, None]
    y = np.arange(MASK_W)[None, :]
    rel = (dj - y).astype(np.float64)
    win = (rel <= 0) & (rel >= -W)

    f16 = np.float16
    in_maps = []
    for core in range(8):
        b, p = divmod(core, 2)
        heads = [2 * s + p for s in range(HPC)]
        hperm = np.concatenate([g * D + perm for g in heads])
        hplain = np.concatenate([g * D + np.arange(D) for g in heads])
        xb = x[:, b, :]
        xT_m = np.ascontiguousarray(xb.T).reshape(NT_C, 128, L)
        wq_m = np.ascontiguousarray(
            wq[hperm].reshape(NT_HD, 128, NT_C, 128).transpose(0, 3, 2, 1)).reshape(NT_HD, 128, C)
        wk_m = np.ascontiguousarray(
            wk_full[hperm].reshape(NT_HD, 128, NT_C, 128).transpose(0, 3, 2, 1)).reshape(NT_HD, 128, C)
        wv_m = wv_full[hplain].T.reshape(NT_C, 128, GD)
        # [q, c, j, which, 1024]: which=0 x tokens, which=1 wv head-dims
        xwv_m = np.ascontiguousarray(np.stack(
            [xT_m.reshape(4, 4, 128, L), wv_m.reshape(4, 4, 128, GD)],
            axis=3).transpose(0, 2, 1, 3, 4))
        wo_m = np.ascontiguousarray(wo[:, hplain].T).reshape(NT_HD, 128, C)
        bq_m = np.ascontiguousarray(bq[hperm].reshape(NT_HD, 128).T)
        bk_m = np.ascontiguousarray(bk_full[hperm].reshape(NT_HD, 128).T)
        expb = np.zeros((HPC, 128, MASK_W), f16)
        for s in range(HPC):
            sl = float(alibi[heads[s]])
            expb[s] = np.where(win, np.exp(sl * rel), 0.0).astype(f16)
        in_maps.append(dict(
            xwv=xwv_m.astype(f16), wq=wq_m.astype(f16), wk=wk_m.astype(f16),
            wo=wo_m.astype(f16),
            cos2=cos2, sin2=sin2, bq=bq_m, bk=bk_m, expb=expb,
            ones=np.ones((128, 128), f16)))
    return in_maps


def gather(results, inputs):
    wo = np.asarray(inputs["wo"], np.float32)
    bo = np.asarray(inputs["bo"], np.float32)
    bv = np.asarray(inputs["bkv"], np.float32)[C:]
    bo_eff = bo + wo @ bv          # p sums to 1, so +bv rides through attn
    out = np.empty((L, N, C), np.float32)
    for b in range(N):
        out[:, b, :] = results[2 * b]["out"] + results[2 * b + 1]["out"] + bo_eff[None, :]
    return out


# ----------------------------------------------------------------------------
# Public entry point: kernel(**inputs) -> (L, N, C) float32
# ----------------------------------------------------------------------------
_NC_CACHE = {}


def _get_nc():
    if "nc" not in _NC_CACHE:
        _NC_CACHE["nc"] = build_nc()
    return _NC_CACHE["nc"]


def kernel(**inputs):
    from concourse import bass_utils
    nc = _get_nc()
    in_maps = marshal(inputs)
    res = bass_utils.run_bass_kernel_spmd(nc, in_maps, core_ids=list(range(8)))
    return gather(res.results, inputs)


# revision 27
# speedup vs baseline: 1.0212x; 1.0212x over previous
"""Trainium2 Bass kernel for sliding-window causal MHA with RoPE + ALiBi.

Sharding: 8 cores = 4 batches x 2 head-sets. Head-sets interleave parity
(core parity p takes global heads p, p+2, ..., p+14) so both per-core
programs have identical attention tile counts after ALiBi-decay window
truncation (steep-slope heads attend far fewer than W keys).

Per-core device program, all matmuls fp16:
  A: v-proj, n-outer accumulation (PE stays fed during the x/wv DMA fill)
  B: q/k-proj + RoPE (Act evac w/ bias, DVE fp16 rope at 2x rate)
  C: per query-group gi (128 queries), two half-passes of 4 heads:
     scores for a descending-j0 span -> one Act exp -> one DVE mask-mul
     (expb master tile: ALiBi weight * window mask, contiguous slice),
     then PV + ones-sums accumulation packed 4 heads/bank, DVE
     reciprocal + normalize. Truncated j-span per head slot via T_PAT.
  D: out-proj interleaved one query-group behind C (fills exp latency),
     partial over the head set; host sums partials + bo + wo@bv.
"""
import sys
sys.path.insert(0, '/opt/trn_rl_repo')
from contextlib import ExitStack

import numpy as np
import concourse.bass as bass
import concourse.bacc as bacc
import concourse.mybir as mybir
import concourse.tile as tile

L, N, C, H, D, W = 1024, 4, 2048, 16, 128, 512
HPC = 8                       # head slots per core
GD = HPC * D                  # 1024 head-dims per core
SCALE = 1.0 / float(np.sqrt(D))
F32 = mybir.dt.float32
F16 = mybir.dt.float16
AF = mybir.ActivationFunctionType
NT_C = C // 128               # 16 contraction tiles over embed dim
NT_HD = GD // 128             # 8 head tiles (1 head each, D=128)
NT_T = L // 128               # 8 token tiles
MASK_W = 640                  # expb master width: y = di + (i0-j0), T<=512
# Truncated window per head slot (parity-max so both core programs match).
# Slot s holds global head 2s+p; slope(s,p)=2^{-(2s+p+1)/2}. T chosen so
# dropped softmax mass <~ e^-8 relative even for the shallower parity.
T_PAT = [32, 64, 128, 128, 256, 512, 512, 512]


def jtiles(s, gi):
    """Descending j0 list for head-slot s, query group [128*gi, 128*gi+128)."""
    i0 = gi * 128
    lo = max(0, i0 - T_PAT[s]) // 128 * 128
    return list(range(i0, lo - 1, -128))


def chunks(lst, n=4):
    return [lst[i:i + n] for i in range(0, len(lst), n)]


def emit(tc, t):
    nc = tc.nc
    cpool = tc.alloc_tile_pool(name="const", bufs=1, side="left")
    cos2 = cpool.tile([128, L], F16, tag="cos2")
    sin2 = cpool.tile([128, L], F16, tag="sin2")
    bq_s = cpool.tile([128, NT_HD], F32, tag="bq")
    bk_s = cpool.tile([128, NT_HD], F32, tag="bk")
    ones = cpool.tile([128, 128], F16, tag="ones")

    # long-lived (left stack): v tiles, then q/k tiles
    vp = tc.alloc_tile_pool(name="vp", bufs=1, side="left")
    vts = [vp.tile([128, GD], F16, tag=f"v{tt}", name=f"v{tt}") for tt in range(NT_T)]

    # single PSUM pool: 8 bank-tags handed across phases with zero
    # pool-transition stalls (WAR deps per tag do the synchronization)
    psp = tc.alloc_tile_pool(name="psp", bufs=1, space="PSUM")

    def bank(i, width=512):
        return psp.tile([128, width], F32, tag=f"b{i}", name=f"b{i}")

    # ---------------- phase A: v-proj (n-outer, 4 passes of 4 groups) -------
    # fill bandwidth: x on the SP queue, wv on the DVE queue in parallel so
    # (x_n, wv_n) pairs arrive faster than the PE consumes them
    xp = tc.alloc_tile_pool(name="xp", bufs=1, side="right")
    expb = [cpool.tile([128, MASK_W], F16, tag=f"eb{s}", name=f"eb{s}")
            for s in range(HPC)]
    wots = [cpool.tile([128, C], F16, tag=f"wo{s}", name=f"wo{s}")
            for s in range(NT_HD)]
    w0p = tc.alloc_tile_pool(name="w0p", bufs=1, side="right")
    # x and wv live interleaved in 4 n-quarter tiles [128, 4 n, 2, 1024]
    # (which=0 -> x tokens, which=1 -> wv head-dims); one DMA delivers the
    # (x token-half, wv gd-half) pair each A-pass-1 n-tile needs, in chunks
    # sized so arrivals outpace PE consumption
    xwv = [xp.tile([128, 4, 2, L], F16, tag=f"x{q}", name=f"x{q}")
           for q in range(4)]

    def xsl(n, a, b):
        return xwv[n // 4][:, n % 4, 0, a:b]

    def wsl(n, a, b):
        return xwv[n // 4][:, n % 4, 1, a:b]

    def fill_chunk(q, ja, jb):
        nc.sync.dma_start(xwv[q][:, ja:jb, :, 0:768],
                          t["xwv"][q][:, ja:jb, :, 0:768])

    nc.sync.dma_start(ones[:], t["ones"][:])
    if True:
        # pass 1 covers 6 of 8 groups, so PE demand per n-tile (1278ns)
        # far exceeds the chunk supply rate (~910ns): robust to jitter
        for j in range(4):
            fill_chunk(0, j, j + 1)
        for q in range(1, 4):
            fill_chunk(q, 0, 2)
            fill_chunk(q, 2, 4)
        for q in range(4):
            nc.sync.dma_start(xwv[q][:, :, 0, 768:1024],
                              t["xwv"][q][:, :, 0, 768:1024])
        for q in range(4):
            nc.sync.dma_start(xwv[q][:, :, 1, 768:1024],
                              t["xwv"][q][:, :, 1, 768:1024])
        # small consts after the fill-critical stream
        nc.sync.dma_start(cos2[:], t["cos2"][:])
        nc.sync.dma_start(sin2[:], t["sin2"][:])
        nc.sync.dma_start(bq_s[:], t["bq"][:])
        nc.sync.dma_start(bk_s[:], t["bk"][:])
        # m=0 q/k weights ahead of the expb/wo bulk so B can start on time
        wt0 = {}
        for wname in ("wq", "wk"):
            wt0[wname] = w0p.tile([128, C], F16, tag=f"{wname}0", name=f"{wname}0")
            nc.sync.dma_start(wt0[wname][:], t[wname][0])
        for s in range(HPC):
            nc.sync.dma_start(expb[s][:], t["expb"][s])
        for s in range(NT_HD):
            nc.sync.dma_start(wots[s][:], t["wo"][s])
        wu = bank(7)
        for w in range(10):
            nc.tensor.matmul(wu[:, 0:128], ones[:], ones[:, 0:128],
                             start=True, stop=True)
        for i2 in range(2):
            for tts, bs in ((range(0, 6), (0, 1, 2, 3, 4, 5)),
                            (range(6, 8), (6, 7))):
                groups = [(tt, i2) for tt in tts]
                pss = [bank(bs[gidx]) for gidx in range(len(groups))]
                for n in range(NT_C):
                    for gidx, (tt, i2g) in enumerate(groups):
                        nc.tensor.matmul(
                            pss[gidx][:],
                            xsl(n, tt * 128, (tt + 1) * 128),
                            wsl(n, i2g * 512, (i2g + 1) * 512),
                            start=(n == 0), stop=(n == NT_C - 1))
                for gidx, (tt, i2g) in enumerate(groups):
                    nc.scalar.activation(
                        vts[tt][:, i2g * 512:(i2g + 1) * 512], pss[gidx][:],
                        AF.Identity, scale=1.0)

    # ---------------- phase B: q/k-proj + rope ----------------
    qkp = tc.alloc_tile_pool(name="qkp", bufs=1, side="left")
    qts = [qkp.tile([128, L], F16, tag=f"q{m}", name=f"q{m}") for m in range(NT_HD)]
    kts = [qkp.tile([128, L], F16, tag=f"k{m}", name=f"k{m}") for m in range(NT_HD)]
    g0p = tc.alloc_tile_pool(name="g0p", bufs=1, side="left")
    awp = tc.alloc_tile_pool(name="awp", bufs=2, side="left")
    cw = tc.alloc_tile_pool(name="cw", bufs=3, side="left")
    pre_pts = {0: {}, 1: {}}
    precnt = [0]
    aw_tiles = {}
    acc01 = {}

    def pv_sums(s, ls, attn2, sums2, pts):
        tiles = [(j0, pT, ci)
                 for chunk, pT in pts[s]
                 for ci, j0 in enumerate(chunk)]
        for ti, (j0, pT, ci) in enumerate(tiles):
            nc.tensor.matmul(
                attn2[:, ls * 128:(ls + 1) * 128],
                vts[j0 // 128][:, s * 128:(s + 1) * 128],
                pT[:, ci * 128:(ci + 1) * 128],
                start=(ti == 0), stop=(ti == len(tiles) - 1))
            nc.tensor.matmul(
                sums2[:, ls * 128:(ls + 1) * 128],
                ones[:],
                pT[:, ci * 128:(ci + 1) * 128],
                start=(ti == 0), stop=(ti == len(tiles) - 1))

    def quarter_norm(gi, qp, attn2, sums2):
        rec = cw.tile([128, 256], F32, tag="rec", name="rec")
        nc.vector.reciprocal(rec[:], sums2)
        awq = awp.tile([128, 256], F16, tag=f"aw{qp}", name=f"aw{qp}")
        nc.vector.tensor_mul(awq[:], attn2, rec[:])
        aw_tiles.setdefault(gi, [None] * 4)[qp] = awq

    def gi0_pv(qp):
        if "ab" not in acc01:
            acc01["ab"] = bank(2)
            acc01["sb"] = bank(3)
        hsl = slice((qp % 2) * 256, (qp % 2) * 256 + 256)
        attn2, sums2 = acc01["ab"][:, hsl], acc01["sb"][:, hsl]
        for ls, s in enumerate((2 * qp, 2 * qp + 1)):
            pv_sums(s, ls, attn2, sums2, pre_pts[0])
        quarter_norm(0, qp, attn2, sums2)

    def pre_scores(s, gi):
        # gi=0/1 scores emitted during B so their exp chains hide under B's
        # matmuls; banks 6/7 are free of B's rotation (0..5)
        i0 = gi * 128
        pre_pts[gi][s] = []
        for chunk in chunks(jtiles(s, gi)):
            ck = len(chunk)
            s_ps = bank(6 + precnt[0] % 2, width=ck * 128)
            precnt[0] += 1
            for ci, j0 in enumerate(chunk):
                nc.tensor.matmul(s_ps[:, ci * 128:(ci + 1) * 128],
                                 kts[s][:, j0:j0 + 128], qts[s][:, i0:i0 + 128],
                                 start=True, stop=True)
            e = g0p.tile([128, ck * 128], F16, tag=f"pe{precnt[0] % 2}",
                         name=f"pe{precnt[0] % 2}")
            nc.scalar.activation(e[:], s_ps[:], AF.Exp, scale=SCALE)
            pT = g0p.tile([128, ck * 128], F16, tag=f"pT{gi}{s}", name=f"pT{gi}{s}")
            c0 = (i0 - chunk[0]) // 128
            nc.vector.tensor_mul(pT[:], e[:], expb[s][:, c0 * 128:(c0 + ck) * 128])
            pre_pts[gi][s].append((chunk, pT))

    bcnt = [0]
    with tc.tile_pool(name="ws", bufs=2, side="right") as ws, \
         tc.tile_pool(name="rp", bufs=3, side="right") as rp:
        for m in range(NT_HD):
            for wname, dst, bias_s in (("wq", qts, bq_s), ("wk", kts, bk_s)):
                if m == 0:
                    wt = wt0[wname]
                else:
                    wt = ws.tile([128, C], F16, tag="wqk", name="wqk")
                    nc.sync.dma_start(wt[:], t[wname][m])
                for i2 in range(2):
                    ps = bank((0, 1, 4, 5)[bcnt[0] % 4])
                    bcnt[0] += 1
                    for n in range(NT_C):
                        nc.tensor.matmul(
                            ps[:],
                            wt[:, n * 128:(n + 1) * 128],
                            xsl(n, i2 * 512, (i2 + 1) * 512),
                            start=(n == 0), stop=(n == NT_C - 1))
                    csl = slice(i2 * 512, (i2 + 1) * 512)
                    qw = rp.tile([128, 512], F16, tag="qw", name="qw")
                    nc.scalar.activation(
                        qw[:], ps[:],
                        AF.Identity, bias=bias_s[:, m:m + 1], scale=1.0)
                    # rope: dst = qw*cos2 + swap_halves(qw)*sin2, all fp16
                    rot = rp.tile([128, 512], F16, tag="rot", name="rot")
                    nc.vector.tensor_copy(rot[0:64, :], qw[64:128, :])
                    nc.vector.tensor_copy(rot[64:128, :], qw[0:64, :])
                    t1 = rp.tile([128, 512], F16, tag="t1", name="t1")
                    nc.vector.tensor_mul(t1[:], qw[:], cos2[:, csl])
                    nc.vector.tensor_mul(rot[:], rot[:], sin2[:, csl])
                    nc.vector.tensor_add(dst[m][:, csl], t1[:], rot[:])
            if m >= 1:
                pre_scores(m - 1, 0)
                pre_scores(m - 1, 1)
            if m in (3, 5, 7):
                gi0_pv((m - 3) // 2)
    w0p.release()
    xp.release()

    # ---------------- phase C+D: attention + out-proj, interleaved ----------
    sccnt = [0]
    dcnt = [0]
    with tc.tile_pool(name="og", bufs=3, side="right") as og:

        def d_chain(tt, cc):
            ps = bank(6 + dcnt[0] % 2)
            dcnt[0] += 1
            for hh in range(NT_HD):
                aw = aw_tiles[tt][hh // 2]
                ls = hh % 2
                nc.tensor.matmul(
                    ps[:],
                    aw[:, ls * 128:(ls + 1) * 128],
                    wots[hh][:, cc * 512:(cc + 1) * 512],
                    start=(hh == 0), stop=(hh == NT_HD - 1))
            o = og.tile([128, 512], F32, tag="o", name="o")
            nc.scalar.activation(o[:], ps[:], AF.Identity, scale=1.0)
            nc.sync.dma_start(
                t["out"][tt * 128:(tt + 1) * 128, cc * 512:(cc + 1) * 512], o[:])

        def scores_for(s, gi, pts, banks=(0, 1, 4, 5)):
            i0 = gi * 128
            pts[s] = []
            for chunk in chunks(jtiles(s, gi)):
                ck = len(chunk)
                s_ps = bank(banks[sccnt[0] % len(banks)], width=ck * 128)
                sccnt[0] += 1
                for ci, j0 in enumerate(chunk):
                    nc.tensor.matmul(
                        s_ps[:, ci * 128:(ci + 1) * 128],
                        kts[s][:, j0:j0 + 128],
                        qts[s][:, i0:i0 + 128],
                        start=True, stop=True)
                e = cw.tile([128, ck * 128], F16, tag="e", name="e")
                nc.scalar.activation(e[:], s_ps[:], AF.Exp, scale=SCALE)
                pT = cw.tile([128, ck * 128], F16, tag="pT", name="pT")
                c0 = (i0 - chunk[0]) // 128
                nc.vector.tensor_mul(
                    pT[:], e[:], expb[s][:, c0 * 128:(c0 + ck) * 128])
                pts[s].append((chunk, pT))



        def gi1_qp(qp, with_d=True):
            hsl = slice((qp % 2) * 256, (qp % 2) * 256 + 256)
            attn2, sums2 = acc01["ab"][:, hsl], acc01["sb"][:, hsl]
            if with_d:
                d_chain(0, qp)
            for ls, s in enumerate((2 * qp, 2 * qp + 1)):
                pv_sums(s, ls, attn2, sums2, pre_pts[1])
            quarter_norm(1, qp, attn2, sums2)

        # C start: gi0-qp3 and slot-7 pre-scores were not possible during B
        # (rope(7) lands after B's last chain); interleave them behind gi1's
        # first quarters so the PE never waits on them directly
        gi1_qp(0, with_d=False)
        gi1_qp(1, with_d=False)
        pre_scores(7, 0)
        pre_scores(7, 1)
        gi0_pv(3)
        gi1_qp(2, with_d=False)
        d_chain(0, 0)
        gi1_qp(3, with_d=False)
        d_chain(0, 1)
        d_chain(0, 2)
        d_chain(0, 3)

        # quarter-passes of 2 heads; even/odd quarters use the two halves of
        # one attn bank (b2) and one sums bank (b3) -- groups in a bank stay
        # sequential, never concurrently open -- freeing b4/b5 for scores
        for gi in range(2, NT_T):
            ab = bank(2)
            sb = bank(3)
            for qp in range(4):
                hpair = (2 * qp, 2 * qp + 1)
                hsl = slice((qp % 2) * 256, (qp % 2) * 256 + 256)
                attn2 = ab[:, hsl]
                sums2 = sb[:, hsl]
                pts = {}
                scores_for(hpair[0], gi, pts)
                scores_for(hpair[1], gi, pts)
                d_chain(gi - 1, qp)
                for ls, s in enumerate(hpair):
                    pv_sums(s, ls, attn2, sums2, pts)
                quarter_norm(gi, qp, attn2, sums2)
        for cc in range(3):
            d_chain(NT_T - 1, cc)
        # final chain split in two halves so the tail evac/DMA starts earlier
        for half in range(2):
            ps = bank(6 + half)
            for hh in range(NT_HD):
                aw = aw_tiles[NT_T - 1][hh // 2]
                nc.tensor.matmul(
                    ps[:, 0:256],
                    aw[:, (hh % 2) * 128:(hh % 2 + 1) * 128],
                    wots[hh][:, 1536 + half * 256:1536 + half * 256 + 256],
                    start=(hh == 0), stop=(hh == NT_HD - 1))
            o = og.tile([128, 256], F32, tag="o", name="o")
            nc.scalar.activation(o[:], ps[:, 0:256], AF.Identity, scale=1.0)
            nc.sync.dma_start(
                t["out"][(NT_T - 1) * 128:NT_T * 128,
                         1536 + half * 256:1536 + half * 256 + 256], o[:])

    psp.release()
    cw.release()
    awp.release()
    g0p.release()
    qkp.release()
    vp.release()
    cpool.release()


def build_nc(enable_asserts=False):
    nc = bacc.Bacc("TRN2", target_bir_lowering=False, debug=False,
                   enable_asserts=enable_asserts, num_devices=8)
    t = {}
    t["xwv"] = nc.dram_tensor("xwv", [4, 128, 4, 2, 1024], F16, kind="ExternalInput").ap()
    t["wq"] = nc.dram_tensor("wq", [NT_HD, 128, C], F16, kind="ExternalInput").ap()
    t["wk"] = nc.dram_tensor("wk", [NT_HD, 128, C], F16, kind="ExternalInput").ap()
    t["wo"] = nc.dram_tensor("wo", [NT_HD, 128, C], F16, kind="ExternalInput").ap()
    t["cos2"] = nc.dram_tensor("cos2", [128, L], F16, kind="ExternalInput").ap()
    t["sin2"] = nc.dram_tensor("sin2", [128, L], F16, kind="ExternalInput").ap()
    t["bq"] = nc.dram_tensor("bq", [128, NT_HD], F32, kind="ExternalInput").ap()
    t["bk"] = nc.dram_tensor("bk", [128, NT_HD], F32, kind="ExternalInput").ap()
    t["expb"] = nc.dram_tensor("expb", [HPC, 128, MASK_W], F16, kind="ExternalInput").ap()
    t["ones"] = nc.dram_tensor("ones", [128, 128], F16, kind="ExternalInput").ap()
    t["out"] = nc.dram_tensor("out", [L, C], F32, kind="ExternalOutput").ap()
    with tile.TileContext(nc) as tc:
        emit(tc, t)
    nc.compile()
    return nc


def marshal(inputs):
    x = np.asarray(inputs["x"], np.float32)
    wq = np.asarray(inputs["wq"], np.float32)
    wkv = np.asarray(inputs["wkv"], np.float32)
    wo = np.asarray(inputs["wo"], np.float32)
    bq = np.asarray(inputs["bq"], np.float32)
    bkv = np.asarray(inputs["bkv"], np.float32)
    alibi = np.asarray(inputs["alibi_slopes"], np.float32)
    wk_full, wv_full = wkv[:C], wkv[C:]
    bk_full = bkv[:C]

    perm = np.concatenate([np.arange(0, D, 2), np.arange(1, D, 2)])

    t_abs = np.arange(W, W + L, dtype=np.float64)
    inv = 1.0 / (10000.0 ** (np.arange(0, D, 2, dtype=np.float64) / D))
    fr = np.outer(t_abs, inv)
    cosT = np.cos(fr).T.astype(np.float32)
    sinT = np.sin(fr).T.astype(np.float32)
    cos2 = np.ascontiguousarray(np.concatenate([cosT, cosT], 0)).astype(np.float16)
    sin2 = np.ascontiguousarray(np.concatenate([-sinT, sinT], 0)).astype(np.float16)

    dj = np.arange(128)[:, None]
    y = np.arange(MASK_W)[None, :]
    rel = (dj - y).astype(np.float64)
    win = (rel <= 0) & (rel >= -W)

    f16 = np.float16
    in_maps = []
    for core in range(8):
        b, p = divmod(core, 2)
        heads = [2 * s + p for s in range(HPC)]
        hperm = np.concatenate([g * D + perm for g in heads])
        hplain = np.concatenate([g * D + np.arange(D) for g in heads])
        xb = x[:, b, :]
        xT_m = np.ascontiguousarray(xb.T).reshape(NT_C, 128, L)
        wq_m = np.ascontiguousarray(
            wq[hperm].reshape(NT_HD, 128, NT_C, 128).transpose(0, 3, 2, 1)).reshape(NT_HD, 128, C)
        wk_m = np.ascontiguousarray(
            wk_full[hperm].reshape(NT_HD, 128, NT_C, 128).transpose(0, 3, 2, 1)).reshape(NT_HD, 128, C)
        wv_m = wv_full[hplain].T.reshape(NT_C, 128, GD)
        # [q, c, j, which, 1024]: which=0 x tokens, which=1 wv head-dims
        xwv_m = np.ascontiguousarray(np.stack(
            [xT_m.reshape(4, 4, 128, L), wv_m.reshape(4, 4, 128, GD)],
            axis=3).transpose(0, 2, 1, 3, 4))
        wo_m = np.ascontiguousarray(wo[:, hplain].T).reshape(NT_HD, 128, C)
        bq_m = np.ascontiguousarray(bq[hperm].reshape(NT_HD, 128).T)
        bk_m = np.ascontiguousarray(bk_full[hperm].reshape(NT_HD, 128).T)
        expb = np.zeros((HPC, 128, MASK_W), f16)
        for s in range(HPC):
            sl = float(alibi[heads[s]])
            expb[s] = np.where(win, np.exp(sl * rel), 0.0).astype(f16)
        in_maps.append(dict(
            xwv=xwv_m.astype(f16), wq=wq_m.astype(f16), wk=wk_m.astype(f16),
            wo=wo_m.astype(f16),
            cos2=cos2, sin2=sin2, bq=bq_m, bk=bk_m, expb=expb,
            ones=np.ones((128, 128), f16)))
    return in_maps


def gather(results, inputs):
    wo = np.asarray(inputs["wo"], np.float32)
    bo = np.asarray(inputs["bo"], np.float32)
    bv = np.asarray(inputs["bkv"], np.float32)[C:]
    bo_eff = bo + wo @ bv          # p sums to 1, so +bv rides through attn
    out = np.empty((L, N, C), np.float32)
    for b in range(N):
        out[:, b, :] = results[2 * b]["out"] + results[2 * b + 1]["out"] + bo_eff[None, :]
    return out


# ----------------------------------------------------------------------------
# Public entry point: kernel(**inputs) -> (L, N, C) float32
# ----------------------------------------------------------------------------
_NC_CACHE = {}


def _get_nc():
    if "nc" not in _NC_CACHE:
        _NC_CACHE["nc"] = build_nc()
    return _NC_CACHE["nc"]


def kernel(**inputs):
    from concourse import bass_utils
    nc = _get_nc()
    in_maps = marshal(inputs)
    res = bass_utils.run_bass_kernel_spmd(nc, in_maps, core_ids=list(range(8)))
    return gather(res.results, inputs)
